# revision 13
# baseline (speedup 1.0000x reference)
"""DogeCDMoME (product-key MoE routing) Trainium2 kernel.

Sharding: pure data-parallel over tokens across 8 NeuronCores (256 tokens
each); dense weights and both embedding tables are replicated per core, so
no collectives are needed.  Inside a core the 256 tokens are processed in
two 128-token passes so that the gather/combine tail of pass 1 overlaps the
dense matmuls of pass 2.

Precision: the routing chain (x -> U -> h -> q -> sim -> top-k) is extremely
sensitive to noise (bf16-level noise flips ~11% of expert selections), so
every matmul runs in native fp32 on the PE (4 cycles/row).

Per-pass pipeline (128 tokens, M=128 matmul tiles):
  A:  U = x @ W_up           (token-major, lhsT = x^T chunks, N=512 slices)
      A = silu(U)            (ScalarE, from PSUM)  -> transpose -> A^T blocks
  B:  h = A @ W_down         (interleaved with A at s-block granularity)
  C:  q = h @ W_q            -> transpose -> q^T
  D:  sim[p,h] = q_slice @ keys[h,:,p,:]^T   (per (plane, head))
  TopK: nc.vector.max / max_index give top-8 directly; cartesian 8x8
      combine via per-partition scalar adds; top-8-of-64 again with max;
      expert ids recovered with a one-hot dot against the 64 combined ids.
  G:  indirect-DMA gather of up_embed rows, g = <h, ue> via
      tensor_tensor_reduce; gates = silu(g) * softmax(scores);
      out = sum_s gate_s * down_embed[idx_s] via scalar_tensor_tensor
      multiply-accumulate; DMA out.
"""

import numpy as np
from contextlib import ExitStack

import concourse.bass as bass
import concourse.mybir as mybir
import concourse.tile as tile
from concourse.bass import IndirectOffsetOnAxis
from concourse.masks import make_identity

AF = mybir.ActivationFunctionType
ALU = mybir.AluOpType
DT = mybir.dt

N_CORES = 8
T_TOTAL = 2048
T_CORE = T_TOTAL // N_CORES      # 256
TT = 128                         # tokens per pass (matmul M tile)
N_PASS = T_CORE // TT            # 2
D = 2048                         # model dim
S = 8192                         # FFN hidden
P = 1024                         # value dim
CQ = 4096                        # W_q output dim = 2*H*(P//2)
H = 4                            # heads
NK = 128                         # keys per plane
TK = 8                           # top-k
E = 16384                        # experts

F32 = DT.float32

# The walrus build in this environment rejects instructions carrying more
# than one attached sync wait ("Too many sync wait commands",
# setupSyncWait<...S3_LW/CTRL...>).  Tile freely attaches several waits to
# one instruction, so after scheduling we hoist all-but-one wait of every
# non-DMA instruction onto standalone EventSemaphore (pure wait)
# instructions inserted right before it on the same engine — semantically
# identical, since engine streams execute in order.
_WAIT_EXEMPT = {"InstEventSemaphore"}


def _legalize_waits(nc, keep=1):
    import re

    n_fix = 0
    for f in nc.m.functions:
        for bb in f.blocks:
            il = bb.instructions
            i = 0
            while i < len(il):
                ins = il[i]
                tname = type(ins).__name__
                # EVENT_SEMAPHORE_RANGE_CLEAR (opcode 176): this walrus build
                # rejects its encoding ("ISA wrong length") — replace with
                # per-semaphore immediate writes of 0.
                if tname == "InstISA" and getattr(ins, "isa_opcode", None) == 176:
                    m = re.search(r"range_first=(\d+) range_last=(\d+)",
                                  ins.concise())
                    lo, hi = int(m.group(1)), int(m.group(2))
                    il.pop(i)
                    del nc.inst_map[ins.name]
                    for k, sem in enumerate(range(lo, hi + 1)):
                        clr = mybir.InstEventSemaphore(
                            name=f"{ins.name}_clr{k}",
                            engine=ins.engine,
                            ins=[],
                            outs=[],
                            sync_info=mybir.SyncInfo(
                                on_wait=list(ins.sync_info.on_wait)
                                if ins.sync_info and k == 0 else [],
                                on_update=[mybir.SyncUpdate(
                                    sync_type="semaphore", id=sem,
                                    ant_name=f"clr{sem}",
                                    update_mode="sem-wr-imm", update_value=0,
                                )],
                            ),
                        )
                        nc.inst_map[clr.name] = clr
                        il.insert(i + k, clr)
                    i += hi - lo + 1
                    continue
                si = ins.sync_info
                waits = list(si.on_wait) if si is not None and si.on_wait else []
                if tname not in _WAIT_EXEMPT and len(waits) > keep:
                    extra, kept = waits[:-keep], waits[-keep:]
                    for k, w in enumerate(extra):
                        nop = mybir.InstEventSemaphore(
                            name=f"{ins.name}_wfix{k}",
                            engine=ins.engine,
                            ins=[],
                            outs=[],
                            sync_info=mybir.SyncInfo(on_wait=[w], on_update=[]),
                        )
                        nc.inst_map[nop.name] = nop
                        il.insert(i, nop)
                        i += 1
                        n_fix += 1
                    ins.sync_info = mybir.SyncInfo(
                        on_wait=kept, on_update=list(si.on_update or [])
                    )
                i += 1
    return n_fix


def build_bass(reps=1):
    nc = bass.Bass(trn_type="TRN2")

    x_d = nc.dram_tensor("x", [T_CORE, D], F32, kind="ExternalInput")
    wup_d = nc.dram_tensor("W_up", [D, S], F32, kind="ExternalInput")
    wdn_d = nc.dram_tensor("W_down", [S, P], F32, kind="ExternalInput")
    wq_d = nc.dram_tensor("W_q", [P, CQ], F32, kind="ExternalInput")
    keys_d = nc.dram_tensor("keys", [H, NK, 2, P // 2], F32, kind="ExternalInput")
    ue_d = nc.dram_tensor("up_embed", [E, P], F32, kind="ExternalInput")
    de_d = nc.dram_tensor("down_embed", [E, D], F32, kind="ExternalInput")
    out_d = nc.dram_tensor("out", [T_CORE, D], F32, kind="ExternalOutput")

    with tile.TileContext(nc) as tc, ExitStack() as ctx:
        # ---------------- pools ----------------
        cpool = ctx.enter_context(tc.tile_pool(name="const", bufs=1))
        xpool = ctx.enter_context(tc.tile_pool(name="xsb", bufs=1))
        xtpool = ctx.enter_context(tc.tile_pool(name="xt", bufs=1))
        wup_pool = ctx.enter_context(tc.tile_pool(name="wup", bufs=2))
        wdn_pool = ctx.enter_context(tc.tile_pool(name="wdn", bufs=2))
        wq_pool = ctx.enter_context(tc.tile_pool(name="wq", bufs=2))
        apool = ctx.enter_context(tc.tile_pool(name="ablk", bufs=1))
        atpool = ctx.enter_context(tc.tile_pool(name="atblk", bufs=2))
        hpool = ctx.enter_context(tc.tile_pool(name="hsb", bufs=2))
        htpool = ctx.enter_context(tc.tile_pool(name="ht", bufs=1))
        qpool = ctx.enter_context(tc.tile_pool(name="qhalf", bufs=1))
        qtpool = ctx.enter_context(tc.tile_pool(name="qt", bufs=1))
        simpool = ctx.enter_context(tc.tile_pool(name="sim", bufs=1))
        tkpool = ctx.enter_context(tc.tile_pool(name="topk", bufs=2))
        gpool = ctx.enter_context(tc.tile_pool(name="gat", bufs=2))
        ue_pool = ctx.enter_context(tc.tile_pool(name="ue", bufs=2))
        de_pool = ctx.enter_context(tc.tile_pool(name="de", bufs=2))
        accpool = ctx.enter_context(tc.tile_pool(name="acc", bufs=1))

        ps_a = ctx.enter_context(tc.tile_pool(name="ps_a", bufs=1, space="PSUM"))
        ps_b = ctx.enter_context(tc.tile_pool(name="ps_b", bufs=1, space="PSUM"))
        ps_tr = ctx.enter_context(tc.tile_pool(name="ps_tr", bufs=2, space="PSUM"))

        # ---------------- constants ----------------
        ident = cpool.tile([128, 128], F32, tag="ident")
        make_identity(nc, ident[:])

        iota_i = cpool.tile([128, 64], DT.int32, tag="iota_i")
        nc.gpsimd.iota(iota_i[:], pattern=[[1, 64]], base=0, channel_multiplier=0)
        iota_f = cpool.tile([128, 64], F32, tag="iota_f")
        nc.vector.tensor_copy(iota_f[:], iota_i[:])

        # keys, transposed: keysT[:, ((p*H+h)*4+dc)*128 : +128] = keys[h,:,p,dc]^T
        keysT = cpool.tile([128, 2 * H * 4 * 128], F32, tag="keysT")
        for p in range(2):
            for hh in range(H):
                for dc in range(4):
                    kst = cpool.tile([128, 128], F32, tag="kstage")
                    nc.sync.dma_start(
                        kst[:], keys_d[hh, :, p, dc * 128:(dc + 1) * 128]
                    )
                    ptr = ps_tr.tile([128, 128], F32, tag="tr")
                    nc.tensor.transpose(ptr[:], kst[:], ident[:])
                    col = ((p * H + hh) * 4 + dc) * 128
                    nc.vector.tensor_copy(keysT[:, col:col + 128], ptr[:])

        # ---------------- per-pass pipeline ----------------
        import contextlib
        rep_ctx = tc.For_i(0, reps, 1) if reps > 1 else contextlib.nullcontext()
        with rep_ctx:
            _pipeline(nc, tc, locals())

    _legalize_waits(nc)
    return nc


def _pipeline(nc, tc, env):
    (x_d, wup_d, wdn_d, wq_d, ue_d, de_d, out_d) = (
        env["x_d"], env["wup_d"], env["wdn_d"], env["wq_d"],
        env["ue_d"], env["de_d"], env["out_d"])
    (xpool, xtpool, wup_pool, wdn_pool, wq_pool, apool, atpool, hpool,
     htpool, qpool, qtpool, simpool, tkpool, gpool, ue_pool, de_pool,
     accpool, ps_a, ps_b, ps_tr) = (
        env["xpool"], env["xtpool"], env["wup_pool"], env["wdn_pool"],
        env["wq_pool"], env["apool"], env["atpool"], env["hpool"],
        env["htpool"], env["qpool"], env["qtpool"], env["simpool"],
        env["tkpool"], env["gpool"], env["ue_pool"], env["de_pool"],
        env["accpool"], env["ps_a"], env["ps_b"], env["ps_tr"])
    ident, iota_f, keysT = env["ident"], env["iota_f"], env["keysT"]

    if True:
        for tt in range(N_PASS):
            # ---- load + transpose x tile ----
            x_sb = xpool.tile([128, D], F32, tag="x_sb")
            nc.sync.dma_start(x_sb[:], x_d[tt * TT:(tt + 1) * TT, :])
            x_t = xtpool.tile([128, D], F32, tag="x_t")  # [d_chunk -> 128 cols each]
            for dc in range(D // 128):
                ptr = ps_tr.tile([128, 128], F32, tag="tr")
                nc.tensor.transpose(ptr[:], x_sb[:, dc * 128:(dc + 1) * 128], ident[:])
                nc.vector.tensor_copy(x_t[:, dc * 128:(dc + 1) * 128], ptr[:])

            # ---- stages A+B interleaved over s-blocks of 2048 ----
            hs = [ps_b.tile([128, 512], F32, tag=f"hs{ss}", name=f"hs{ss}") for ss in range(2)]
            for sb in range(4):
                us = [ps_a.tile([128, 512], F32, tag=f"us{ss}", name=f"us{ss}") for ss in range(4)]
                for dc in range(D // 128):
                    wup_t = wup_pool.tile([128, 2048], F32, tag="wup_t")
                    nc.sync.dma_start(
                        wup_t[:],
                        wup_d[dc * 128:(dc + 1) * 128, sb * 2048:(sb + 1) * 2048],
                    )
                    for ss in range(4):
                        nc.tensor.matmul(
                            us[ss][:],
                            lhsT=x_t[:, dc * 128:(dc + 1) * 128],
                            rhs=wup_t[:, ss * 512:(ss + 1) * 512],
                            start=(dc == 0),
                            stop=(dc == D // 128 - 1),
                        )
                a_blk = apool.tile([128, 2048], F32, tag="a_blk")
                for ss in range(4):
                    sl = slice(ss * 512, (ss + 1) * 512)
                    nc.scalar.activation(a_blk[:, sl], us[ss][:], AF.Sigmoid)
                    nc.vector.tensor_tensor(
                        out=a_blk[:, sl], in0=a_blk[:, sl], in1=us[ss][:],
                        op=ALU.mult,
                    )
                at_blk = atpool.tile([128, 2048], F32, tag="at_blk")
                for j in range(16):
                    ptr = ps_tr.tile([128, 128], F32, tag="tr")
                    nc.tensor.transpose(
                        ptr[:], a_blk[:, j * 128:(j + 1) * 128], ident[:]
                    )
                    nc.vector.tensor_copy(at_blk[:, j * 128:(j + 1) * 128], ptr[:])
                # stage B for this block's 16 s-chunks
                for j in range(16):
                    sc = sb * 16 + j
                    wdn_t = wdn_pool.tile([128, 1024], F32, tag="wdn_t")
                    nc.sync.dma_start(wdn_t[:], wdn_d[sc * 128:(sc + 1) * 128, :])
                    for ss in range(2):
                        nc.tensor.matmul(
                            hs[ss][:],
                            lhsT=at_blk[:, j * 128:(j + 1) * 128],
                            rhs=wdn_t[:, ss * 512:(ss + 1) * 512],
                            start=(sc == 0),
                            stop=(sc == S // 128 - 1),
                        )

            h_sb = hpool.tile([128, P], F32, tag="h_sb")
            for ss in range(2):
                nc.vector.tensor_copy(h_sb[:, ss * 512:(ss + 1) * 512], hs[ss][:])
            h_t = htpool.tile([128, P], F32, tag="h_t")
            for j in range(P // 128):
                ptr = ps_tr.tile([128, 128], F32, tag="tr")
                nc.tensor.transpose(ptr[:], h_sb[:, j * 128:(j + 1) * 128], ident[:])
                nc.vector.tensor_copy(h_t[:, j * 128:(j + 1) * 128], ptr[:])

            # ---- stage C: q = h @ W_q ----
            q_t = qtpool.tile([128, CQ], F32, tag="q_t")
            for ch in range(2):
                qs = [ps_a.tile([128, 512], F32, tag=f"us{ss}", name=f"qs{ss}") for ss in range(4)]
                for pc in range(P // 128):
                    wq_t = wq_pool.tile([128, 2048], F32, tag="wq_t")
                    nc.sync.dma_start(
                        wq_t[:],
                        wq_d[pc * 128:(pc + 1) * 128, ch * 2048:(ch + 1) * 2048],
                    )
                    for ss in range(4):
                        nc.tensor.matmul(
                            qs[ss][:],
                            lhsT=h_t[:, pc * 128:(pc + 1) * 128],
                            rhs=wq_t[:, ss * 512:(ss + 1) * 512],
                            start=(pc == 0),
                            stop=(pc == P // 128 - 1),
                        )
                q_half = qpool.tile([128, 2048], F32, tag="q_half")
                for ss in range(4):
                    nc.vector.tensor_copy(
                        q_half[:, ss * 512:(ss + 1) * 512], qs[ss][:]
                    )
                for j in range(16):
                    ptr = ps_tr.tile([128, 128], F32, tag="tr")
                    nc.tensor.transpose(
                        ptr[:], q_half[:, j * 128:(j + 1) * 128], ident[:]
                    )
                    col = ch * 2048 + j * 128
                    nc.vector.tensor_copy(q_t[:, col:col + 128], ptr[:])

            # ---- stage D: sim[p][:, h*128:+128] ----
            sim_p = [simpool.tile([128, H * NK], F32, tag=f"sim{p}", name=f"sim{p}") for p in range(2)]
            for p in range(2):
                for hh in range(H):
                    ptr = ps_tr.tile([128, 128], F32, tag="tr")
                    for dc in range(4):
                        nc.tensor.matmul(
                            ptr[:],
                            lhsT=q_t[:, p * 2048 + hh * 512 + dc * 128:
                                     p * 2048 + hh * 512 + (dc + 1) * 128],
                            rhs=keysT[:, ((p * H + hh) * 4 + dc) * 128:
                                      ((p * H + hh) * 4 + dc + 1) * 128],
                            start=(dc == 0),
                            stop=(dc == 3),
                        )
                    nc.vector.tensor_copy(
                        sim_p[p][:, hh * NK:(hh + 1) * NK], ptr[:]
                    )

            # ---- top-k + gather + combine ----
            g_all = gpool.tile([128, H * TK], F32, tag="g_all")
            gate_all = gpool.tile([128, H * TK], F32, tag="gate_all")
            eidx_f = gpool.tile([128, H * TK], F32, tag="eidx_f")

            for hh in range(H):
                sx = tkpool.tile([128, 8], F32, tag="sx")
                sy = tkpool.tile([128, 8], F32, tag="sy")
                ix = tkpool.tile([128, 8], DT.uint32, tag="ix")
                iy = tkpool.tile([128, 8], DT.uint32, tag="iy")
                simx = sim_p[0][:, hh * NK:(hh + 1) * NK]
                simy = sim_p[1][:, hh * NK:(hh + 1) * NK]
                nc.vector.max(sx[:], simx)
                nc.vector.max_index(ix[:], sx[:], simx)
                nc.vector.max(sy[:], simy)
                nc.vector.max_index(iy[:], sy[:], simy)

                ixf = tkpool.tile([128, 8], F32, tag="ixf")
                iyf = tkpool.tile([128, 8], F32, tag="iyf")
                nc.vector.tensor_copy(ixf[:], ix[:])
                nc.vector.tensor_copy(iyf[:], iy[:])
                cix = tkpool.tile([128, 8], F32, tag="cix")
                nc.vector.tensor_scalar_mul(cix[:], ixf[:], float(NK))

                allsc = tkpool.tile([128, 64], F32, tag="allsc")
                allid = tkpool.tile([128, 64], F32, tag="allid")
                for i in range(8):
                    nc.vector.tensor_scalar_add(
                        allsc[:, i * 8:(i + 1) * 8], sy[:], sx[:, i:i + 1]
                    )
                    nc.vector.tensor_scalar_add(
                        allid[:, i * 8:(i + 1) * 8], iyf[:], cix[:, i:i + 1]
                    )

                msc = tkpool.tile([128, 8], F32, tag="msc")
                pos = tkpool.tile([128, 8], DT.uint32, tag="pos")
                nc.vector.max(msc[:], allsc[:])
                nc.vector.max_index(pos[:], msc[:], allsc[:])
                posf = tkpool.tile([128, 8], F32, tag="posf")
                nc.vector.tensor_copy(posf[:], pos[:])

                oh = tkpool.tile([128, 64], F32, tag="oh")
                ohscr = tkpool.tile([128, 64], F32, tag="ohscr")
                for s in range(8):
                    nc.vector.tensor_scalar(
                        oh[:], iota_f[:], posf[:, s:s + 1], None, op0=ALU.is_equal
                    )
                    nc.vector.scalar_tensor_tensor(
                        out=ohscr[:],
                        in0=oh[:],
                        scalar=1.0,
                        in1=allid[:],
                        op0=ALU.bypass,
                        op1=ALU.mult,
                        accum_out=eidx_f[:, hh * TK + s:hh * TK + s + 1],
                    )

                # softmax over the 8 scores
                rmax = tkpool.tile([128, 1], F32, tag="rmax")
                nc.vector.tensor_reduce(
                    rmax[:], msc[:], axis=mybir.AxisListType.X, op=ALU.max
                )
                nrmax = tkpool.tile([128, 1], F32, tag="nrmax")
                nc.vector.tensor_scalar_mul(nrmax[:], rmax[:], -1.0)
                esc = tkpool.tile([128, 8], F32, tag="esc")
                ssum = tkpool.tile([128, 1], F32, tag="ssum")
                nc.scalar.activation(
                    esc[:], msc[:], AF.Exp, bias=nrmax[:, :], accum_out=ssum[:]
                )
                rinv = tkpool.tile([128, 1], F32, tag="rinv")
                nc.vector.reciprocal(rinv[:], ssum[:])
                nc.vector.tensor_scalar_mul(
                    gate_all[:, hh * TK:(hh + 1) * TK], esc[:], rinv[:, :]
                )

            ei32 = gpool.tile([128, H * TK], DT.int32, tag="ei32")
            nc.vector.tensor_copy(ei32[:], eidx_f[:])

            # gather up_embed rows, compute g
            gscr = gpool.tile([128, P], F32, tag="gscr")
            for s in range(H * TK):
                ue_t = ue_pool.tile([128, P], F32, tag="ue_t")
                nc.gpsimd.indirect_dma_start(
                    out=ue_t[:],
                    out_offset=None,
                    in_=ue_d[:],
                    in_offset=IndirectOffsetOnAxis(ap=ei32[:, s:s + 1], axis=0),
                )
                nc.vector.scalar_tensor_tensor(
                    out=gscr[:],
                    in0=ue_t[:],
                    scalar=1.0,
                    in1=h_sb[:],
                    op0=ALU.bypass,
                    op1=ALU.mult,
                    accum_out=g_all[:, s:s + 1],
                )

            gsig = gpool.tile([128, H * TK], F32, tag="gsig")
            nc.scalar.activation(gsig[:], g_all[:], AF.Sigmoid)
            gsil = gpool.tile([128, H * TK], F32, tag="gsil")
            nc.vector.tensor_tensor(
                out=gsil[:], in0=gsig[:], in1=g_all[:], op=ALU.mult
            )
            w_all = gpool.tile([128, H * TK], F32, tag="w_all")
            nc.vector.tensor_tensor(
                out=w_all[:], in0=gsil[:], in1=gate_all[:], op=ALU.mult
            )

            # gather down_embed rows, weighted accumulate
            acc = accpool.tile([128, D], F32, tag="acc")
            nc.vector.memset(acc[:], 0.0)
            for s in range(H * TK):
                de_t = de_pool.tile([128, D], F32, tag="de_t")
                nc.gpsimd.indirect_dma_start(
                    out=de_t[:],
                    out_offset=None,
                    in_=de_d[:],
                    in_offset=IndirectOffsetOnAxis(ap=ei32[:, s:s + 1], axis=0),
                )
                nc.vector.scalar_tensor_tensor(
                    out=acc[:],
                    in0=de_t[:],
                    scalar=w_all[:, s:s + 1],
                    in1=acc[:],
                    op0=ALU.mult,
                    op1=ALU.add,
                )
            nc.sync.dma_start(out_d[tt * TT:(tt + 1) * TT, :], acc[:])


_NC_CACHE = None


def _get_nc():
    global _NC_CACHE
    if _NC_CACHE is None:
        _NC_CACHE = build_bass()
    return _NC_CACHE


def kernel(hidden_states, W_up, W_down, W_q, keys, up_embed, down_embed):
    from concourse.bass_utils import run_bass_kernel_spmd

    x = np.ascontiguousarray(
        np.asarray(hidden_states, dtype=np.float32).reshape(T_TOTAL, D)
    )
    shared = {
        "W_up": np.ascontiguousarray(np.asarray(W_up, dtype=np.float32)),
        "W_down": np.ascontiguousarray(np.asarray(W_down, dtype=np.float32)),
        "W_q": np.ascontiguousarray(np.asarray(W_q, dtype=np.float32)),
        "keys": np.ascontiguousarray(np.asarray(keys, dtype=np.float32)),
        "up_embed": np.ascontiguousarray(np.asarray(up_embed, dtype=np.float32)),
        "down_embed": np.ascontiguousarray(np.asarray(down_embed, dtype=np.float32)),
    }
    in_maps = [
        {"x": np.ascontiguousarray(x[c * T_CORE:(c + 1) * T_CORE]), **shared}
        for c in range(N_CORES)
    ]
    nc = _get_nc()
    res = run_bass_kernel_spmd(nc, in_maps, list(range(N_CORES))).results
    out = np.concatenate([res[c]["out"] for c in range(N_CORES)], axis=0)
    return out.reshape(1, T_TOTAL, D)


# revision 15
# speedup vs baseline: 1.2424x; 1.2424x over previous
"""DogeCDMoME (product-key MoE routing) Trainium2 kernel.

Sharding: pure data-parallel over tokens across 8 NeuronCores (256 tokens
each); dense weights and both embedding tables are replicated per core, so
no collectives are needed.  Inside a core the 256 tokens are processed in
two 128-token passes so that the gather/combine tail of pass 1 overlaps the
dense matmuls of pass 2.

Precision: the routing chain (x -> U -> h -> q -> sim -> top-k) is extremely
sensitive to noise (bf16-level noise flips ~11% of expert selections), so
every matmul runs in native fp32 on the PE (4 cycles/row).

Per-pass pipeline (128 tokens, M=128 matmul tiles):
  A:  U = x @ W_up           (token-major, lhsT = x^T chunks, N=512 slices)
      A = silu(U)            (ScalarE, from PSUM)  -> transpose -> A^T blocks
  B:  h = A @ W_down         (interleaved with A at s-block granularity)
  C:  q = h @ W_q            -> transpose -> q^T
  D:  sim[p,h] = q_slice @ keys[h,:,p,:]^T   (per (plane, head))
  TopK: nc.vector.max / max_index give top-8 directly; cartesian 8x8
      combine via per-partition scalar adds; top-8-of-64 again with max;
      expert ids recovered with a one-hot dot against the 64 combined ids.
  G:  indirect-DMA gather of up_embed rows, g = <h, ue> via
      tensor_tensor_reduce; gates = silu(g) * softmax(scores);
      out = sum_s gate_s * down_embed[idx_s] via scalar_tensor_tensor
      multiply-accumulate; DMA out.
"""

import numpy as np
from contextlib import ExitStack

import concourse.bass as bass
import concourse.mybir as mybir
import concourse.tile as tile
from concourse.bass import IndirectOffsetOnAxis
from concourse.masks import make_identity

AF = mybir.ActivationFunctionType
ALU = mybir.AluOpType
DT = mybir.dt

N_CORES = 8
T_TOTAL = 2048
T_CORE = T_TOTAL // N_CORES      # 256
TT = 128                         # tokens per pass (matmul M tile)
N_PASS = T_CORE // TT            # 2
D = 2048                         # model dim
S = 8192                         # FFN hidden
P = 1024                         # value dim
CQ = 4096                        # W_q output dim = 2*H*(P//2)
H = 4                            # heads
NK = 128                         # keys per plane
TK = 8                           # top-k
E = 16384                        # experts

F32 = DT.float32

# CoreSim lacks the Silu activation; hardware has it.  sim_test sets this to
# True to build the sigmoid+multiply equivalent instead.
SIM_COMPAT = False

# The walrus build in this environment rejects instructions carrying more
# than one attached sync wait ("Too many sync wait commands",
# setupSyncWait<...S3_LW/CTRL...>).  Tile freely attaches several waits to
# one instruction, so after scheduling we hoist all-but-one wait of every
# non-DMA instruction onto standalone EventSemaphore (pure wait)
# instructions inserted right before it on the same engine — semantically
# identical, since engine streams execute in order.
_WAIT_EXEMPT = {"InstEventSemaphore"}


def _legalize_waits(nc, keep=1):
    import re

    n_fix = 0
    for f in nc.m.functions:
        for bb in f.blocks:
            il = bb.instructions
            i = 0
            while i < len(il):
                ins = il[i]
                tname = type(ins).__name__
                # EVENT_SEMAPHORE_RANGE_CLEAR (opcode 176): this walrus build
                # rejects its encoding ("ISA wrong length") — replace with
                # per-semaphore immediate writes of 0.
                if tname == "InstISA" and getattr(ins, "isa_opcode", None) == 176:
                    m = re.search(r"range_first=(\d+) range_last=(\d+)",
                                  ins.concise())
                    lo, hi = int(m.group(1)), int(m.group(2))
                    il.pop(i)
                    del nc.inst_map[ins.name]
                    for k, sem in enumerate(range(lo, hi + 1)):
                        clr = mybir.InstEventSemaphore(
                            name=f"{ins.name}_clr{k}",
                            engine=ins.engine,
                            ins=[],
                            outs=[],
                            sync_info=mybir.SyncInfo(
                                on_wait=list(ins.sync_info.on_wait)
                                if ins.sync_info and k == 0 else [],
                                on_update=[mybir.SyncUpdate(
                                    sync_type="semaphore", id=sem,
                                    ant_name=f"clr{sem}",
                                    update_mode="sem-wr-imm", update_value=0,
                                )],
                            ),
                        )
                        nc.inst_map[clr.name] = clr
                        il.insert(i + k, clr)
                    i += hi - lo + 1
                    continue
                si = ins.sync_info
                waits = list(si.on_wait) if si is not None and si.on_wait else []
                if tname not in _WAIT_EXEMPT and len(waits) > keep:
                    extra, kept = waits[:-keep], waits[-keep:]
                    for k, w in enumerate(extra):
                        nop = mybir.InstEventSemaphore(
                            name=f"{ins.name}_wfix{k}",
                            engine=ins.engine,
                            ins=[],
                            outs=[],
                            sync_info=mybir.SyncInfo(on_wait=[w], on_update=[]),
                        )
                        nc.inst_map[nop.name] = nop
                        il.insert(i, nop)
                        i += 1
                        n_fix += 1
                    ins.sync_info = mybir.SyncInfo(
                        on_wait=kept, on_update=list(si.on_update or [])
                    )
                i += 1
    return n_fix


def build_bass(reps=1):
    nc = bass.Bass(trn_type="TRN2")

    x_d = nc.dram_tensor("x", [T_CORE, D], F32, kind="ExternalInput")
    wup_d = nc.dram_tensor("W_up", [D, S], F32, kind="ExternalInput")
    wdn_d = nc.dram_tensor("W_down", [S, P], F32, kind="ExternalInput")
    wq_d = nc.dram_tensor("W_q", [P, CQ], F32, kind="ExternalInput")
    keys_d = nc.dram_tensor("keys", [H, NK, 2, P // 2], F32, kind="ExternalInput")
    ue_d = nc.dram_tensor("up_embed", [E, P], F32, kind="ExternalInput")
    de_d = nc.dram_tensor("down_embed", [E, D], F32, kind="ExternalInput")
    out_d = nc.dram_tensor("out", [T_CORE, D], F32, kind="ExternalOutput")

    with tile.TileContext(nc) as tc, ExitStack() as ctx:
        # ---------------- pools ----------------
        cpool = ctx.enter_context(tc.tile_pool(name="const", bufs=1))
        xpool = ctx.enter_context(tc.tile_pool(name="xsb", bufs=1))
        xtpool = ctx.enter_context(tc.tile_pool(name="xt", bufs=1))
        wup_pool = ctx.enter_context(tc.tile_pool(name="wup", bufs=2))
        wdn_pool = ctx.enter_context(tc.tile_pool(name="wdn", bufs=2))
        wq_pool = ctx.enter_context(tc.tile_pool(name="wq", bufs=2))
        apool = ctx.enter_context(tc.tile_pool(name="ablk", bufs=1))
        atpool = ctx.enter_context(tc.tile_pool(name="atblk", bufs=2))
        hpool = ctx.enter_context(tc.tile_pool(name="hsb", bufs=2))
        htpool = ctx.enter_context(tc.tile_pool(name="ht", bufs=1))
        qpool = ctx.enter_context(tc.tile_pool(name="qhalf", bufs=1))
        qtpool = ctx.enter_context(tc.tile_pool(name="qt", bufs=1))
        simpool = ctx.enter_context(tc.tile_pool(name="sim", bufs=1))
        tkpool = ctx.enter_context(tc.tile_pool(name="topk", bufs=2))
        gpool = ctx.enter_context(tc.tile_pool(name="gat", bufs=2))
        ue_pool = ctx.enter_context(tc.tile_pool(name="ue", bufs=2))
        de_pool = ctx.enter_context(tc.tile_pool(name="de", bufs=2))
        accpool = ctx.enter_context(tc.tile_pool(name="acc", bufs=1))

        ps_a = ctx.enter_context(tc.tile_pool(name="ps_a", bufs=1, space="PSUM"))
        ps_b = ctx.enter_context(tc.tile_pool(name="ps_b", bufs=1, space="PSUM"))
        ps_tr = ctx.enter_context(tc.tile_pool(name="ps_tr", bufs=2, space="PSUM"))

        # ---------------- constants ----------------
        ident = cpool.tile([128, 128], F32, tag="ident")
        make_identity(nc, ident[:])

        iota_i = cpool.tile([128, 64], DT.int32, tag="iota_i")
        nc.gpsimd.iota(iota_i[:], pattern=[[1, 64]], base=0, channel_multiplier=0)
        iota_f = cpool.tile([128, 64], F32, tag="iota_f")
        nc.vector.tensor_copy(iota_f[:], iota_i[:])

        # keys, transposed: keysT[:, ((p*H+h)*4+dc)*128 : +128] = keys[h,:,p,dc]^T
        keysT = cpool.tile([128, 2 * H * 4 * 128], F32, tag="keysT")
        for p in range(2):
            for hh in range(H):
                for dc in range(4):
                    kst = cpool.tile([128, 128], F32, tag="kstage")
                    nc.sync.dma_start(
                        kst[:], keys_d[hh, :, p, dc * 128:(dc + 1) * 128]
                    )
                    ptr = ps_tr.tile([128, 128], F32, tag="tr")
                    nc.tensor.transpose(ptr[:], kst[:], ident[:])
                    col = ((p * H + hh) * 4 + dc) * 128
                    nc.scalar.copy(keysT[:, col:col + 128], ptr[:])

        # ---------------- per-pass pipeline ----------------
        # reps>1 unrolls the whole pipeline for benchmarking (amortizes the
        # ~77 ms axon dispatch overhead over several kernel executions).
        for _rep in range(reps):
            _pipeline(nc, tc, locals())

    _legalize_waits(nc)
    return nc


def _pipeline(nc, tc, env):
    (x_d, wup_d, wdn_d, wq_d, ue_d, de_d, out_d) = (
        env["x_d"], env["wup_d"], env["wdn_d"], env["wq_d"],
        env["ue_d"], env["de_d"], env["out_d"])
    (xpool, xtpool, wup_pool, wdn_pool, wq_pool, apool, atpool, hpool,
     htpool, qpool, qtpool, simpool, tkpool, gpool, ue_pool, de_pool,
     accpool, ps_a, ps_b, ps_tr) = (
        env["xpool"], env["xtpool"], env["wup_pool"], env["wdn_pool"],
        env["wq_pool"], env["apool"], env["atpool"], env["hpool"],
        env["htpool"], env["qpool"], env["qtpool"], env["simpool"],
        env["tkpool"], env["gpool"], env["ue_pool"], env["de_pool"],
        env["accpool"], env["ps_a"], env["ps_b"], env["ps_tr"])
    ident, iota_f, keysT = env["ident"], env["iota_f"], env["keysT"]

    if True:
        for tt in range(N_PASS):
            # ---- load + transpose x tile ----
            x_sb = xpool.tile([128, D], F32, tag="x_sb")
            nc.sync.dma_start(x_sb[:], x_d[tt * TT:(tt + 1) * TT, :])
            x_t = xtpool.tile([128, D], F32, tag="x_t")  # [d_chunk -> 128 cols each]
            for dc in range(D // 128):
                ptr = ps_tr.tile([128, 128], F32, tag="tr")
                nc.tensor.transpose(ptr[:], x_sb[:, dc * 128:(dc + 1) * 128], ident[:])
                nc.scalar.copy(x_t[:, dc * 128:(dc + 1) * 128], ptr[:])

            # ---- stages A+B interleaved over s-blocks of 2048 ----
            hs = [ps_b.tile([128, 512], F32, tag=f"hs{ss}", name=f"hs{ss}") for ss in range(2)]
            for sb in range(4):
                us = [ps_a.tile([128, 512], F32, tag=f"us{ss}", name=f"us{ss}") for ss in range(4)]
                for dc in range(D // 128):
                    wup_t = wup_pool.tile([128, 2048], F32, tag="wup_t")
                    nc.sync.dma_start(
                        wup_t[:],
                        wup_d[dc * 128:(dc + 1) * 128, sb * 2048:(sb + 1) * 2048],
                    )
                    for ss in range(4):
                        nc.tensor.matmul(
                            us[ss][:],
                            lhsT=x_t[:, dc * 128:(dc + 1) * 128],
                            rhs=wup_t[:, ss * 512:(ss + 1) * 512],
                            start=(dc == 0),
                            stop=(dc == D // 128 - 1),
                        )
                a_blk = apool.tile([128, 2048], F32, tag="a_blk")
                for ss in range(4):
                    sl = slice(ss * 512, (ss + 1) * 512)
                    if SIM_COMPAT:
                        nc.scalar.activation(a_blk[:, sl], us[ss][:], AF.Sigmoid)
                        nc.vector.tensor_tensor(
                            out=a_blk[:, sl], in0=a_blk[:, sl], in1=us[ss][:],
                            op=ALU.mult,
                        )
                    else:
                        # single ScalarE op keeps the PSUM-release chain off
                        # the (busy, in-order) VectorE
                        nc.scalar.activation(a_blk[:, sl], us[ss][:], AF.Silu)
                at_blk = atpool.tile([128, 2048], F32, tag="at_blk")
                for j in range(16):
                    ptr = ps_tr.tile([128, 128], F32, tag="tr")
                    nc.tensor.transpose(
                        ptr[:], a_blk[:, j * 128:(j + 1) * 128], ident[:]
                    )
                    nc.scalar.copy(at_blk[:, j * 128:(j + 1) * 128], ptr[:])
                # stage B for this block's 16 s-chunks
                for j in range(16):
                    sc = sb * 16 + j
                    wdn_t = wdn_pool.tile([128, 1024], F32, tag="wdn_t")
                    nc.sync.dma_start(wdn_t[:], wdn_d[sc * 128:(sc + 1) * 128, :])
                    for ss in range(2):
                        nc.tensor.matmul(
                            hs[ss][:],
                            lhsT=at_blk[:, j * 128:(j + 1) * 128],
                            rhs=wdn_t[:, ss * 512:(ss + 1) * 512],
                            start=(sc == 0),
                            stop=(sc == S // 128 - 1),
                        )

            h_sb = hpool.tile([128, P], F32, tag="h_sb")
            for ss in range(2):
                nc.scalar.copy(h_sb[:, ss * 512:(ss + 1) * 512], hs[ss][:])
            h_t = htpool.tile([128, P], F32, tag="h_t")
            for j in range(P // 128):
                ptr = ps_tr.tile([128, 128], F32, tag="tr")
                nc.tensor.transpose(ptr[:], h_sb[:, j * 128:(j + 1) * 128], ident[:])
                nc.scalar.copy(h_t[:, j * 128:(j + 1) * 128], ptr[:])

            # ---- stage C: q = h @ W_q ----
            q_t = qtpool.tile([128, CQ], F32, tag="q_t")
            for ch in range(2):
                qs = [ps_a.tile([128, 512], F32, tag=f"us{ss}", name=f"qs{ss}") for ss in range(4)]
                for pc in range(P // 128):
                    wq_t = wq_pool.tile([128, 2048], F32, tag="wq_t")
                    nc.sync.dma_start(
                        wq_t[:],
                        wq_d[pc * 128:(pc + 1) * 128, ch * 2048:(ch + 1) * 2048],
                    )
                    for ss in range(4):
                        nc.tensor.matmul(
                            qs[ss][:],
                            lhsT=h_t[:, pc * 128:(pc + 1) * 128],
                            rhs=wq_t[:, ss * 512:(ss + 1) * 512],
                            start=(pc == 0),
                            stop=(pc == P // 128 - 1),
                        )
                q_half = qpool.tile([128, 2048], F32, tag="q_half")
                for ss in range(4):
                    nc.scalar.copy(
                        q_half[:, ss * 512:(ss + 1) * 512], qs[ss][:]
                    )
                for j in range(16):
                    ptr = ps_tr.tile([128, 128], F32, tag="tr")
                    nc.tensor.transpose(
                        ptr[:], q_half[:, j * 128:(j + 1) * 128], ident[:]
                    )
                    col = ch * 2048 + j * 128
                    nc.scalar.copy(q_t[:, col:col + 128], ptr[:])

            # ---- stage D: sim[p][:, h*128:+128] ----
            sim_p = [simpool.tile([128, H * NK], F32, tag=f"sim{p}", name=f"sim{p}") for p in range(2)]
            for p in range(2):
                for hh in range(H):
                    ptr = ps_tr.tile([128, 128], F32, tag="tr")
                    for dc in range(4):
                        nc.tensor.matmul(
                            ptr[:],
                            lhsT=q_t[:, p * 2048 + hh * 512 + dc * 128:
                                     p * 2048 + hh * 512 + (dc + 1) * 128],
                            rhs=keysT[:, ((p * H + hh) * 4 + dc) * 128:
                                      ((p * H + hh) * 4 + dc + 1) * 128],
                            start=(dc == 0),
                            stop=(dc == 3),
                        )
                    nc.scalar.copy(
                        sim_p[p][:, hh * NK:(hh + 1) * NK], ptr[:]
                    )

            # ---- top-k + gather + combine ----
            g_all = gpool.tile([128, H * TK], F32, tag="g_all")
            gate_all = gpool.tile([128, H * TK], F32, tag="gate_all")
            eidx_f = gpool.tile([128, H * TK], F32, tag="eidx_f")

            for hh in range(H):
                sx = tkpool.tile([128, 8], F32, tag="sx")
                sy = tkpool.tile([128, 8], F32, tag="sy")
                ix = tkpool.tile([128, 8], DT.uint32, tag="ix")
                iy = tkpool.tile([128, 8], DT.uint32, tag="iy")
                simx = sim_p[0][:, hh * NK:(hh + 1) * NK]
                simy = sim_p[1][:, hh * NK:(hh + 1) * NK]
                nc.vector.max(sx[:], simx)
                nc.vector.max_index(ix[:], sx[:], simx)
                nc.vector.max(sy[:], simy)
                nc.vector.max_index(iy[:], sy[:], simy)

                ixf = tkpool.tile([128, 8], F32, tag="ixf")
                iyf = tkpool.tile([128, 8], F32, tag="iyf")
                nc.vector.tensor_copy(ixf[:], ix[:])
                nc.vector.tensor_copy(iyf[:], iy[:])
                cix = tkpool.tile([128, 8], F32, tag="cix")
                nc.vector.tensor_scalar_mul(cix[:], ixf[:], float(NK))

                allsc = tkpool.tile([128, 64], F32, tag="allsc")
                allid = tkpool.tile([128, 64], F32, tag="allid")
                for i in range(8):
                    nc.vector.tensor_scalar_add(
                        allsc[:, i * 8:(i + 1) * 8], sy[:], sx[:, i:i + 1]
                    )
                    nc.vector.tensor_scalar_add(
                        allid[:, i * 8:(i + 1) * 8], iyf[:], cix[:, i:i + 1]
                    )

                msc = tkpool.tile([128, 8], F32, tag="msc")
                pos = tkpool.tile([128, 8], DT.uint32, tag="pos")
                nc.vector.max(msc[:], allsc[:])
                nc.vector.max_index(pos[:], msc[:], allsc[:])
                posf = tkpool.tile([128, 8], F32, tag="posf")
                nc.vector.tensor_copy(posf[:], pos[:])

                oh = tkpool.tile([128, 64], F32, tag="oh")
                ohscr = tkpool.tile([128, 64], F32, tag="ohscr")
                for s in range(8):
                    nc.vector.tensor_scalar(
                        oh[:], iota_f[:], posf[:, s:s + 1], None, op0=ALU.is_equal
                    )
                    nc.vector.scalar_tensor_tensor(
                        out=ohscr[:],
                        in0=oh[:],
                        scalar=1.0,
                        in1=allid[:],
                        op0=ALU.bypass,
                        op1=ALU.mult,
                        accum_out=eidx_f[:, hh * TK + s:hh * TK + s + 1],
                    )

                # softmax over the 8 scores
                rmax = tkpool.tile([128, 1], F32, tag="rmax")
                nc.vector.tensor_reduce(
                    rmax[:], msc[:], axis=mybir.AxisListType.X, op=ALU.max
                )
                nrmax = tkpool.tile([128, 1], F32, tag="nrmax")
                nc.vector.tensor_scalar_mul(nrmax[:], rmax[:], -1.0)
                esc = tkpool.tile([128, 8], F32, tag="esc")
                ssum = tkpool.tile([128, 1], F32, tag="ssum")
                nc.scalar.activation(
                    esc[:], msc[:], AF.Exp, bias=nrmax[:, :], accum_out=ssum[:]
                )
                rinv = tkpool.tile([128, 1], F32, tag="rinv")
                nc.vector.reciprocal(rinv[:], ssum[:])
                nc.vector.tensor_scalar_mul(
                    gate_all[:, hh * TK:(hh + 1) * TK], esc[:], rinv[:, :]
                )

            ei32 = gpool.tile([128, H * TK], DT.int32, tag="ei32")
            nc.vector.tensor_copy(ei32[:], eidx_f[:])

            # gather up_embed rows, compute g
            gscr = gpool.tile([128, P], F32, tag="gscr")
            for s in range(H * TK):
                ue_t = ue_pool.tile([128, P], F32, tag="ue_t")
                nc.gpsimd.indirect_dma_start(
                    out=ue_t[:],
                    out_offset=None,
                    in_=ue_d[:],
                    in_offset=IndirectOffsetOnAxis(ap=ei32[:, s:s + 1], axis=0),
                )
                nc.vector.scalar_tensor_tensor(
                    out=gscr[:],
                    in0=ue_t[:],
                    scalar=1.0,
                    in1=h_sb[:],
                    op0=ALU.bypass,
                    op1=ALU.mult,
                    accum_out=g_all[:, s:s + 1],
                )

            gsig = gpool.tile([128, H * TK], F32, tag="gsig")
            nc.scalar.activation(gsig[:], g_all[:], AF.Sigmoid)
            gsil = gpool.tile([128, H * TK], F32, tag="gsil")
            nc.vector.tensor_tensor(
                out=gsil[:], in0=gsig[:], in1=g_all[:], op=ALU.mult
            )
            w_all = gpool.tile([128, H * TK], F32, tag="w_all")
            nc.vector.tensor_tensor(
                out=w_all[:], in0=gsil[:], in1=gate_all[:], op=ALU.mult
            )

            # gather down_embed rows, weighted accumulate
            acc = accpool.tile([128, D], F32, tag="acc")
            nc.vector.memset(acc[:], 0.0)
            for s in range(H * TK):
                de_t = de_pool.tile([128, D], F32, tag="de_t")
                nc.gpsimd.indirect_dma_start(
                    out=de_t[:],
                    out_offset=None,
                    in_=de_d[:],
                    in_offset=IndirectOffsetOnAxis(ap=ei32[:, s:s + 1], axis=0),
                )
                nc.vector.scalar_tensor_tensor(
                    out=acc[:],
                    in0=de_t[:],
                    scalar=w_all[:, s:s + 1],
                    in1=acc[:],
                    op0=ALU.mult,
                    op1=ALU.add,
                )
            nc.sync.dma_start(out_d[tt * TT:(tt + 1) * TT, :], acc[:])


_NC_CACHE = None


def _get_nc():
    global _NC_CACHE
    if _NC_CACHE is None:
        _NC_CACHE = build_bass()
    return _NC_CACHE


def kernel(hidden_states, W_up, W_down, W_q, keys, up_embed, down_embed):
    from concourse.bass_utils import run_bass_kernel_spmd

    x = np.ascontiguousarray(
        np.asarray(hidden_states, dtype=np.float32).reshape(T_TOTAL, D)
    )
    shared = {
        "W_up": np.ascontiguousarray(np.asarray(W_up, dtype=np.float32)),
        "W_down": np.ascontiguousarray(np.asarray(W_down, dtype=np.float32)),
        "W_q": np.ascontiguousarray(np.asarray(W_q, dtype=np.float32)),
        "keys": np.ascontiguousarray(np.asarray(keys, dtype=np.float32)),
        "up_embed": np.ascontiguousarray(np.asarray(up_embed, dtype=np.float32)),
        "down_embed": np.ascontiguousarray(np.asarray(down_embed, dtype=np.float32)),
    }
    in_maps = [
        {"x": np.ascontiguousarray(x[c * T_CORE:(c + 1) * T_CORE]), **shared}
        for c in range(N_CORES)
    ]
    nc = _get_nc()
    res = run_bass_kernel_spmd(nc, in_maps, list(range(N_CORES))).results
    out = np.concatenate([res[c]["out"] for c in range(N_CORES)], axis=0)
    return out.reshape(1, T_TOTAL, D)
